# revision 64
# baseline (speedup 1.0000x reference)
"""Grouped 3x3 SAME conv on 8 Trainium2 NeuronCores.

Problem: x[16,56,56,256] NHWC, 8 groups of 32->64 channels, 3x3 SAME,
out[16,56,56,512], fp32.

Strategy (hardcoded):
  - Data-parallel over batch: core i handles images [2i, 2i+1].
  - fp16 operands (fp32 PSUM accumulate, rel err ~3e-4). The host lays
    out, per group, the full kh-replicated contraction tile
    [96=(3 kh x 32 c), 6844] where each kh block is the zero-bordered
    padded channel row of BOTH images back-to-back, shifted by 58*kh.
    One 874 KB DMA with 13.7 KB contiguous lines loads a whole group
    (large packets keep the 16 SDMA engines near line rate, and one DMA
    per group avoids thrashing the 8 DMA-completion semaphore lanes).
  - Input DMAs for pair gp+1 issue before pair gp's compute (Sync HWDGE
    queue) so loads overlap compute; weights/bias ride the Scalar HWDGE
    queue; output DMAs ride the GpSimd SWDGE queue. A short block of
    dummy matmuls runs while the first loads land, warming the PE HAM
    clock gate so real matmuls start at 2.4 GHz.
  - Matmul: K=96 per group, the two groups of a pair on PE col-halves
    via tile_position (0,0)/(0,64); the kw shift is a column offset into
    the same SBUF tile. kw-outer / tile-inner order; the 7 spatial tiles
    of an image live in 7 PSUM banks.
  - PSUM->SBUF copy + bias add alternates Vector/Scalar engines, writes
    fp16; one 831 KB output DMA per (pair, image).
"""

import numpy as np

G = 8        # groups
P = 32       # in-channels per group
F = 64       # out-channels per group
H = W = 56
HP = WP = 58           # zero-padded spatial
SP = HP * WP           # 3364 padded pixels
SHIFT = WP             # column shift of one image row
N_CORES = 8
B_PER_CORE = 2
NPAIR = G // 2
CROW = SP + 2 * SHIFT   # 3480: one image's zero-bordered row
CDRAM = B_PER_CORE * CROW       # 6960: both images back-to-back
CVIEW = CROW + SP               # 6844: shifted window over both images
NT = 8 * SHIFT          # 464: spatial tile = 8 padded image rows
NTILE = 7               # covers padded cols [58, 3306)
NOUT = NTILE * NT       # 3248 output cols per image
NWARM = 58

_PROG_CACHE = {}


def _build_program():
    import concourse.bacc as bacc
    import concourse.mybir as mybir
    import concourse.tile as tile

    dt = mybir.dt
    nc = bacc.Bacc(
        "TRN2",
        target_bir_lowering=False,
        debug=False,
        num_devices=N_CORES,
    )

    f32 = dt.float32
    f16 = dt.float16
    IDENT = mybir.ActivationFunctionType.Identity

    # pre-replicated contraction tiles: [g, (kh,c), shifted 2-image row]
    xd = nc.dram_tensor("xd", [G, 3 * P, CVIEW], f16, kind="ExternalInput")
    # dummy destination used to gate each pair load on the previous one
    gate = nc.dram_tensor("gate", [1, 64], f16, kind="ExternalOutput")
    # [p=(kh,c), g, kw, f]
    wd = nc.dram_tensor("wd", [3 * P, G, 3, F], f16, kind="ExternalInput")
    bd = nc.dram_tensor("bd", [2 * F, NPAIR], f32, kind="ExternalInput")
    outT = nc.dram_tensor("outT", [NPAIR, 2 * F, B_PER_CORE, NOUT], f16,
                          kind="ExternalOutput")

    with tile.TileContext(nc) as tc:
        with (
            tc.tile_pool(name="const", bufs=1) as cpool,
            tc.tile_pool(name="xg", bufs=3) as xpool,
            tc.tile_pool(name="ot", bufs=4) as opool,
            tc.tile_pool(name="ps", bufs=1, space="PSUM") as ppool,
        ):
            wsb = cpool.tile([3 * P, G, 3, F], f16)
            nc.scalar.dma_start(wsb[:], wd[:])
            bsb = cpool.tile([2 * F, NPAIR], f32)
            nc.scalar.dma_start(bsb[:], bd[:])

            warm = cpool.tile([1, 128], f16)
            nc.gpsimd.memset(warm[:], 0)
            pw = ppool.tile([1, 128], f32, tag="pw", name="pw")

            def pe_filler(n):
                # dependency-free dummy matmuls: the PE runs them while
                # real matmuls wait on DMA, so the HAM clock gate never
                # sees an idle window and re-throttles to 1.2 GHz
                for _ in range(n):
                    nc.tensor.matmul(pw[:, :], warm[0:1, 0:1], warm[:, :],
                                     start=True, stop=True)

            pe_filler(NWARM)

            def load_x(eng, tag, g, split=False):
                # xa rides the Sync HWDGE ring, xb the Scalar one: HBM
                # reads are latency-bound per SDMA ring (~13 B/ns vs 26
                # for writes), so two rings overlap two read streams, and
                # each ring's FIFO keeps earlier loads completing first.
                xt = xpool.tile([3 * P, CVIEW], f16, tag=tag, name=tag)
                if split:
                    # image-0 half first: the first matmuls only read
                    # cols [0, 3480), so they start after ~1/4 of the
                    # pair's bytes have landed
                    eng.dma_start(xt[:, 0:CROW], xd[g, :, 0:CROW])
                    eng.dma_start(xt[:, CROW:], xd[g, :, CROW:])
                else:
                    eng.dma_start(xt[:], xd[g, :, :])
                return xt

            # Pacing matters more than prefetch depth: with round-robin
            # ring service, every in-flight DMA delays every completion,
            # and completions gate matmuls. xa loads are serialized by a
            # dummy 128 B readback gate on Sync; xb(gp+1) is emitted
            # after img0's combines so the Scalar queue releases it at
            # mid-pair, one load in flight per ring at a time.
            xas = {0: load_x(nc.sync, "xa", 0, split=True)}
            xbs = {0: load_x(nc.scalar, "xb", 1, split=True)}
            for gp in range(NPAIR):
                xa, xb = xas[gp], xbs[gp]
                if gp + 1 < NPAIR:
                    nc.sync.dma_start(gate[:, :], xa[0:1, 0:64])
                    xas[gp + 1] = load_x(nc.sync, "xa", 2 * gp + 2)
                    xbs[gp + 1] = load_x(nc.scalar, "xb", 2 * gp + 3)

                # fill the reproducible DMA-wait stalls (pair-0's second
                # image chunk, the gp2/gp3 load boundaries) with slightly
                # fewer filler matmuls than the observed stall lengths
                if gp == 2:
                    pe_filler(36)
                elif gp == 3:
                    pe_filler(8)
                for img in range(B_PER_CORE):
                    base = CROW * img
                    if gp == 0 and img == 1:
                        pe_filler(18)
                    osb = opool.tile([2 * F, NOUT], f16, tag="osb")
                    pss = [ppool.tile([2 * F, NT], f32, tag=f"ps{t}",
                                      name=f"ps{t}")
                           for t in range(NTILE)]
                    for dw in range(3):
                        for t in range(NTILE):
                            o = base + (1 + 8 * t) * SHIFT - 1 + dw
                            nc.tensor.matmul(
                                pss[t][0:F, :],
                                wsb[:, 2 * gp, dw, :],
                                xa[:, o:o + NT],
                                start=(dw == 0), stop=(dw == 2),
                                tile_position=(0, 0),
                            )
                            nc.tensor.matmul(
                                pss[t][F:2 * F, :],
                                wsb[:, 2 * gp + 1, dw, :],
                                xb[:, o:o + NT],
                                start=(dw == 0), stop=(dw == 2),
                                tile_position=(0, F),
                            )
                    final = (gp == NPAIR - 1 and img == B_PER_CORE - 1)
                    # on the final image, tile 6's combine gates the last
                    # output quarter; give it only two queue predecessors
                    # on the Vector engine instead of three
                    on_vec = ((0, 3, 6) if final else (0, 2, 4, 6))
                    for t in range(NTILE):
                        dst = osb[:, t * NT:(t + 1) * NT]
                        if t in on_vec:
                            nc.vector.tensor_scalar_add(
                                dst, pss[t][:, :], bsb[:, gp:gp + 1])
                        else:
                            nc.scalar.activation(
                                dst, pss[t][:, :], IDENT,
                                bias=bsb[:, gp:gp + 1], scale=1.0)
                    if final:
                        # final output: quarters alternating between the
                        # (now idle) Sync and Scalar HWDGE rings so the
                        # transfers pipeline with the last combines and
                        # the post-last-matmul chain is one combine plus
                        # a 232 KB write
                        nc.sync.dma_start(outT[gp, :, img, 0:2 * NT],
                                          osb[:, 0:2 * NT])
                        nc.scalar.dma_start(outT[gp, :, img, 2 * NT:4 * NT],
                                            osb[:, 2 * NT:4 * NT])
                        nc.sync.dma_start(outT[gp, :, img, 4 * NT:6 * NT],
                                          osb[:, 4 * NT:6 * NT])
                        nc.scalar.dma_start(outT[gp, :, img, 6 * NT:],
                                            osb[:, 6 * NT:])
                    else:
                        nc.gpsimd.dma_start(outT[gp, :, img, :], osb[:])

    nc.compile()
    return nc


def _get_program():
    if "nc" not in _PROG_CACHE:
        _PROG_CACHE["nc"] = _build_program()
    return _PROG_CACHE["nc"]


def prepare_in_maps(x, kernels, bias):
    x = np.ascontiguousarray(x, dtype=np.float32)
    kernels = np.ascontiguousarray(kernels, dtype=np.float32)
    bias = np.ascontiguousarray(bias, dtype=np.float32)
    nb = x.shape[0]

    # zero-bordered padded channel rows, both images of a core
    # back-to-back, then the three kh-shifted views stacked on the
    # partition axis: [core, g, (kh,c), CVIEW]
    xc = x.transpose(0, 3, 1, 2)                       # [nb, 256, 56, 56]
    xpad = np.zeros((nb, G * P, HP, WP), np.float16)
    xpad[:, :, 1:1 + H, 1:1 + W] = xc.astype(np.float16)
    xrow = np.zeros((nb, G, P, CROW), np.float16)
    xrow[:, :, :, SHIFT:SHIFT + SP] = xpad.reshape(nb, G, P, SP)
    xcat = (xrow.reshape(N_CORES, B_PER_CORE, G, P, CROW)
                .transpose(0, 2, 3, 1, 4)
                .reshape(N_CORES, G, P, CDRAM))
    xd = np.empty((N_CORES, G, 3, P, CVIEW), np.float16)
    for kh in range(3):
        xd[:, :, kh, :, :] = xcat[:, :, :, SHIFT * kh:SHIFT * kh + CVIEW]
    xd = xd.reshape(N_CORES, G, 3 * P, CVIEW)

    # [p=(kh,c), g, kw, f]
    wd = np.ascontiguousarray(
        kernels.transpose(1, 3, 0, 2, 4).reshape(3 * P, G, 3, F)
    ).astype(np.float16)

    bd = np.ascontiguousarray(bias.reshape(NPAIR, 2 * F).T)

    return [
        {"xd": np.ascontiguousarray(xd[i]), "wd": wd, "bd": bd}
        for i in range(N_CORES)
    ]


def gather_output(results, nb):
    out = np.empty((nb, H, W, G * F), np.float32)
    for i in range(N_CORES):
        o = results[i]["outT"].astype(np.float32)  # [4, 128, 2, 3248]
        o = o.reshape(NPAIR, 2 * F, B_PER_CORE, H, WP)[:, :, :, :, 1:1 + W]
        out[i * B_PER_CORE:(i + 1) * B_PER_CORE] = (
            o.transpose(2, 3, 4, 0, 1).reshape(B_PER_CORE, H, W, G * F))
    return out


def kernel(x, kernels, bias):
    from concourse.bass_utils import run_bass_kernel_spmd

    nc = _get_program()
    in_maps = prepare_in_maps(x, kernels, bias)
    res = run_bass_kernel_spmd(nc, in_maps, list(range(N_CORES)))
    return gather_output(res.results, np.asarray(x).shape[0])


# revision 68
# speedup vs baseline: 1.0034x; 1.0034x over previous
"""Grouped 3x3 SAME conv on 8 Trainium2 NeuronCores.

Problem: x[16,56,56,256] NHWC, 8 groups of 32->64 channels, 3x3 SAME,
out[16,56,56,512], fp32.

Strategy (hardcoded):
  - Data-parallel over batch: core i handles images [2i, 2i+1].
  - fp16 operands (fp32 PSUM accumulate, rel err ~3e-4). The host lays
    out, per group, the full kh-replicated contraction tile
    [96=(3 kh x 32 c), 6844] where each kh block is the zero-bordered
    padded channel row of BOTH images back-to-back, shifted by 58*kh.
    One 874 KB DMA with 13.7 KB contiguous lines loads a whole group
    (large packets keep the 16 SDMA engines near line rate, and one DMA
    per group avoids thrashing the 8 DMA-completion semaphore lanes).
  - Input DMAs for pair gp+1 issue before pair gp's compute (Sync HWDGE
    queue) so loads overlap compute; weights/bias ride the Scalar HWDGE
    queue; output DMAs ride the GpSimd SWDGE queue. A short block of
    dummy matmuls runs while the first loads land, warming the PE HAM
    clock gate so real matmuls start at 2.4 GHz.
  - Matmul: K=96 per group, the two groups of a pair on PE col-halves
    via tile_position (0,0)/(0,64); the kw shift is a column offset into
    the same SBUF tile. kw-outer / tile-inner order; the 7 spatial tiles
    of an image live in 7 PSUM banks.
  - PSUM->SBUF copy + bias add alternates Vector/Scalar engines, writes
    fp16; one 831 KB output DMA per (pair, image).
"""

import numpy as np

G = 8        # groups
P = 32       # in-channels per group
F = 64       # out-channels per group
H = W = 56
HP = WP = 58           # zero-padded spatial
SP = HP * WP           # 3364 padded pixels
SHIFT = WP             # column shift of one image row
N_CORES = 8
B_PER_CORE = 2
NPAIR = G // 2
CROW = SP + 2 * SHIFT   # 3480: one image's zero-bordered row
CDRAM = B_PER_CORE * CROW       # 6960: both images back-to-back
CVIEW = CROW + SP               # 6844: shifted window over both images
NT = 8 * SHIFT          # 464: spatial tile = 8 padded image rows
NTILE = 7               # covers padded cols [58, 3306)
NOUT = NTILE * NT       # 3248 output cols per image
NWARM = 12

_PROG_CACHE = {}


def _build_program():
    import concourse.bacc as bacc
    import concourse.mybir as mybir
    import concourse.tile as tile

    dt = mybir.dt
    nc = bacc.Bacc(
        "TRN2",
        target_bir_lowering=False,
        debug=False,
        num_devices=N_CORES,
    )

    f32 = dt.float32
    f16 = dt.float16
    IDENT = mybir.ActivationFunctionType.Identity

    # pre-replicated contraction tiles: [g, (kh,c), shifted 2-image row]
    xd = nc.dram_tensor("xd", [G, 3 * P, CVIEW], f16, kind="ExternalInput")
    # dummy destination used to gate each pair load on the previous one
    gate = nc.dram_tensor("gate", [1, 64], f16, kind="ExternalOutput")
    # [p=(kh,c), g, kw, f]
    wd = nc.dram_tensor("wd", [3 * P, G, 3, F], f16, kind="ExternalInput")
    bd = nc.dram_tensor("bd", [2 * F, NPAIR], f32, kind="ExternalInput")
    outT = nc.dram_tensor("outT", [NPAIR, 2 * F, B_PER_CORE, NOUT], f16,
                          kind="ExternalOutput")

    with tile.TileContext(nc) as tc:
        with (
            tc.tile_pool(name="const", bufs=1) as cpool,
            tc.tile_pool(name="xg", bufs=3) as xpool,
            tc.tile_pool(name="ot", bufs=4) as opool,
            tc.tile_pool(name="ps", bufs=1, space="PSUM") as ppool,
        ):
            wsb = cpool.tile([3 * P, G, 3, F], f16)
            nc.scalar.dma_start(wsb[:], wd[:])
            bsb = cpool.tile([2 * F, NPAIR], f32)
            nc.scalar.dma_start(bsb[:], bd[:])

            # full-array warmup matmuls: the HAM clock gate tracks actual
            # PE array activity (K=1 dummies never advanced it — every
            # trace showed the first 8/8 transition at ~24us), so stream
            # K=128 x M=128 x N=512 on a zeroed scratch while the first
            # loads land, and the real matmuls start at 2.4 GHz.
            warm = cpool.tile([2 * F, 640], f16)
            nc.gpsimd.memset(warm[:], 0)
            pw = ppool.tile([2 * F, 512], f32, tag="pw", name="pw")
            for _ in range(NWARM):
                nc.tensor.matmul(pw[:, :], warm[:, 0:2 * F],
                                 warm[:, 2 * F:640], start=True, stop=True)

            def load_x(eng, tag, g, split=False):
                # xa rides the Sync HWDGE ring, xb the Scalar one: HBM
                # reads are latency-bound per SDMA ring (~13 B/ns vs 26
                # for writes), so two rings overlap two read streams, and
                # each ring's FIFO keeps earlier loads completing first.
                xt = xpool.tile([3 * P, CVIEW], f16, tag=tag, name=tag)
                if split:
                    # image-0 half first: the first matmuls only read
                    # cols [0, 3480), so they start after ~1/4 of the
                    # pair's bytes have landed
                    eng.dma_start(xt[:, 0:CROW], xd[g, :, 0:CROW])
                    eng.dma_start(xt[:, CROW:], xd[g, :, CROW:])
                else:
                    eng.dma_start(xt[:], xd[g, :, :])
                return xt

            # Pacing matters more than prefetch depth: with round-robin
            # ring service, every in-flight DMA delays every completion,
            # and completions gate matmuls. xa loads are serialized by a
            # dummy 128 B readback gate on Sync; xb(gp+1) is emitted
            # after img0's combines so the Scalar queue releases it at
            # mid-pair, one load in flight per ring at a time.
            xas = {0: load_x(nc.sync, "xa", 0, split=True)}
            xbs = {0: load_x(nc.scalar, "xb", 1, split=True)}
            for gp in range(NPAIR):
                xa, xb = xas[gp], xbs[gp]
                if gp + 1 < NPAIR:
                    nc.sync.dma_start(gate[:, :], xa[0:1, 0:64])
                    xas[gp + 1] = load_x(nc.sync, "xa", 2 * gp + 2)
                    xbs[gp + 1] = load_x(nc.scalar, "xb", 2 * gp + 3)

                for img in range(B_PER_CORE):
                    base = CROW * img
                    osb = opool.tile([2 * F, NOUT], f16, tag="osb")
                    pss = [ppool.tile([2 * F, NT], f32, tag=f"ps{t}",
                                      name=f"ps{t}")
                           for t in range(NTILE)]
                    for dw in range(3):
                        for t in range(NTILE):
                            o = base + (1 + 8 * t) * SHIFT - 1 + dw
                            nc.tensor.matmul(
                                pss[t][0:F, :],
                                wsb[:, 2 * gp, dw, :],
                                xa[:, o:o + NT],
                                start=(dw == 0), stop=(dw == 2),
                                tile_position=(0, 0),
                            )
                            nc.tensor.matmul(
                                pss[t][F:2 * F, :],
                                wsb[:, 2 * gp + 1, dw, :],
                                xb[:, o:o + NT],
                                start=(dw == 0), stop=(dw == 2),
                                tile_position=(0, F),
                            )
                    final = (gp == NPAIR - 1 and img == B_PER_CORE - 1)
                    # on the final image, tile 6's combine gates the last
                    # output quarter; give it only two queue predecessors
                    # on the Vector engine instead of three
                    on_vec = ((0, 3, 6) if final else (0, 2, 4, 6))
                    for t in range(NTILE):
                        dst = osb[:, t * NT:(t + 1) * NT]
                        if t in on_vec:
                            nc.vector.tensor_scalar_add(
                                dst, pss[t][:, :], bsb[:, gp:gp + 1])
                        else:
                            nc.scalar.activation(
                                dst, pss[t][:, :], IDENT,
                                bias=bsb[:, gp:gp + 1], scale=1.0)
                    if final:
                        # final output: quarters alternating between the
                        # (now idle) Sync and Scalar HWDGE rings so the
                        # transfers pipeline with the last combines and
                        # the post-last-matmul chain is one combine plus
                        # a 232 KB write
                        nc.sync.dma_start(outT[gp, :, img, 0:2 * NT],
                                          osb[:, 0:2 * NT])
                        nc.scalar.dma_start(outT[gp, :, img, 2 * NT:4 * NT],
                                            osb[:, 2 * NT:4 * NT])
                        nc.sync.dma_start(outT[gp, :, img, 4 * NT:6 * NT],
                                          osb[:, 4 * NT:6 * NT])
                        nc.scalar.dma_start(outT[gp, :, img, 6 * NT:],
                                            osb[:, 6 * NT:])
                    else:
                        nc.gpsimd.dma_start(outT[gp, :, img, :], osb[:])

    nc.compile()
    return nc


def _get_program():
    if "nc" not in _PROG_CACHE:
        _PROG_CACHE["nc"] = _build_program()
    return _PROG_CACHE["nc"]


def prepare_in_maps(x, kernels, bias):
    x = np.ascontiguousarray(x, dtype=np.float32)
    kernels = np.ascontiguousarray(kernels, dtype=np.float32)
    bias = np.ascontiguousarray(bias, dtype=np.float32)
    nb = x.shape[0]

    # zero-bordered padded channel rows, both images of a core
    # back-to-back, then the three kh-shifted views stacked on the
    # partition axis: [core, g, (kh,c), CVIEW]
    xc = x.transpose(0, 3, 1, 2)                       # [nb, 256, 56, 56]
    xpad = np.zeros((nb, G * P, HP, WP), np.float16)
    xpad[:, :, 1:1 + H, 1:1 + W] = xc.astype(np.float16)
    xrow = np.zeros((nb, G, P, CROW), np.float16)
    xrow[:, :, :, SHIFT:SHIFT + SP] = xpad.reshape(nb, G, P, SP)
    xcat = (xrow.reshape(N_CORES, B_PER_CORE, G, P, CROW)
                .transpose(0, 2, 3, 1, 4)
                .reshape(N_CORES, G, P, CDRAM))
    xd = np.empty((N_CORES, G, 3, P, CVIEW), np.float16)
    for kh in range(3):
        xd[:, :, kh, :, :] = xcat[:, :, :, SHIFT * kh:SHIFT * kh + CVIEW]
    xd = xd.reshape(N_CORES, G, 3 * P, CVIEW)

    # [p=(kh,c), g, kw, f]
    wd = np.ascontiguousarray(
        kernels.transpose(1, 3, 0, 2, 4).reshape(3 * P, G, 3, F)
    ).astype(np.float16)

    bd = np.ascontiguousarray(bias.reshape(NPAIR, 2 * F).T)

    return [
        {"xd": np.ascontiguousarray(xd[i]), "wd": wd, "bd": bd}
        for i in range(N_CORES)
    ]


def gather_output(results, nb):
    out = np.empty((nb, H, W, G * F), np.float32)
    for i in range(N_CORES):
        o = results[i]["outT"].astype(np.float32)  # [4, 128, 2, 3248]
        o = o.reshape(NPAIR, 2 * F, B_PER_CORE, H, WP)[:, :, :, :, 1:1 + W]
        out[i * B_PER_CORE:(i + 1) * B_PER_CORE] = (
            o.transpose(2, 3, 4, 0, 1).reshape(B_PER_CORE, H, W, G * F))
    return out


def kernel(x, kernels, bias):
    from concourse.bass_utils import run_bass_kernel_spmd

    nc = _get_program()
    in_maps = prepare_in_maps(x, kernels, bias)
    res = run_bass_kernel_spmd(nc, in_maps, list(range(N_CORES)))
    return gather_output(res.results, np.asarray(x).shape[0])
